# revision 37
# baseline (speedup 1.0000x reference)
"""Multi-head attention Trainium2 Bass kernel.

Problem: B=4, S=2048, D=1024, H=16 heads (head_dim 64).
  q = (query @ Wq.T + bq).astype(f16); k, v likewise
  energy = einsum('bhqd,bhkd', q, k) / sqrt(64)   (f16)
  attn = softmax(energy, -1)                       (f16)
  x = einsum('bhqk,bhkd', attn, v).astype(f32)
  out = x @ Wo.T + bo                              (f32)

Sharding (8 cores): core c handles batch b = c//2 and head-group hg = c%2
(8 heads = 512 of the 1024 hidden dims).  QKV projections are column-split,
out-projection is row-split; the two partial outputs per batch are summed on
the host.  Biases: bq/bk are added on-chip (per-partition bias on the DVE
eviction); bv/bo contribute `bv_local @ WoT_local + bo` — a constant row
(softmax rows sum to 1) added on the host.

On-chip dataflow per core (all f16 matmul inputs, f32 PSUM):
  1. QT = WqT_loc.T @ XTq  -> [512, 2048] (d_local on partitions), same KT.
     V = XTv.T @ WvT_loc   -> [2048, 512] (s on partitions), stored per-head
     with an appended ones column (V_aug [128, 8*65]).
  2. Per head pair (row-tiled PE, head0 partitions 0:64 / head1 64:128) and
     q-block of 512: for each k-chunk of 128:
       ST[k,q] scores (transposed layout) into a double-buffered [128, 1024]
       PSUM tile, one ACT exp per chunk (scale=1/8 fused, no max subtraction
       — energies are ~N(0,1) so exp fits f16 comfortably), AV matmul with
       ones-augmented V accumulating O_unnorm.T [65, 512]; row 64 = softmax
       denominator.
  3. Normalize: reciprocal of denom row, gpsimd partition-broadcast,
     DVE multiply -> OT f16 (pair-packed [128, 2048] per d-chunk; odd head
     routed through a base-0 tmp tile + SBUF DMA to partitions 64:128).
  4. Out-projection: Y[q,1024] = sum_t OT_t.T @ WoT_t, f16 out (partials
     summed in f32 on the host; fp16 quantization of the partials is well
     inside the error budget).

Schedule notes (v2): the kernel is paced by ACT exp (~287us) and PE slots
(~345us).  Input DMA is column-split so the first scores start after ~5MB
instead of ~11MB; xtv arrives via a separate (DVE) DMA queue with deep
prefetch; V-projection groups are pulled forward into the DMA-bound head;
the out-projection output is fp16 and leaves via the DVE queue to shorten
the tail.
"""

import numpy as np

B, S, D, H = 4, 2048, 1024, 16
HD = 64
NCORES = 8
DL = 512  # d_local per core
HL = 8  # local heads per core
KC = 8  # contraction chunks (D / 128) for projections
DC = 4  # d_local chunks of 128
SC4 = 4  # S chunks of 512
SC16 = 16  # S chunks of 128
VW = HD + 1  # per-head V width incl. ones column (65)

_PROGRAM = None


def _build_program():
    import concourse.mybir as mybir
    import concourse.tile as tile
    from concourse import bacc

    f16 = mybir.dt.float16
    f32 = mybir.dt.float32
    ACT = mybir.ActivationFunctionType

    nc = bacc.Bacc("TRN2", target_bir_lowering=False, debug=False)

    xtq = nc.declare_dram_parameter("xtq", [D, S], f16, isOutput=False)
    xtk = nc.declare_dram_parameter("xtk", [D, S], f16, isOutput=False)
    # V input pre-arranged on the host so each s-slice of the V-projection
    # stationary is one contiguous [128, 1024] block:
    #   xtvg[sc*128 + p, kc*128 + si] = XTv[kc*128 + p, sc*128 + si]
    xtvg = nc.declare_dram_parameter("xtvg", [S, D], f16, isOutput=False)
    wqt = nc.declare_dram_parameter("wqt", [D, DL], f16, isOutput=False)
    wkt = nc.declare_dram_parameter("wkt", [D, DL], f16, isOutput=False)
    wvt = nc.declare_dram_parameter("wvt", [D, DL], f16, isOutput=False)
    wot = nc.declare_dram_parameter("wot", [DL, D], f16, isOutput=False)
    bq = nc.declare_dram_parameter("bq", [DL], f32, isOutput=False)
    bk = nc.declare_dram_parameter("bk", [DL], f32, isOutput=False)
    y = nc.declare_dram_parameter("y", [S, D], f16, isOutput=True)

    with tile.TileContext(nc) as tc:
        # ---- persistent SBUF pools ----
        with (
            tc.tile_pool(name="wpool", bufs=1) as wpool,
            tc.tile_pool(name="bpool", bufs=1) as bpool,
            tc.tile_pool(name="qkv_sb", bufs=1) as qkv_sb,
            tc.tile_pool(name="ot_sb", bufs=1) as ot_pool,
        ):
            # weights: wx_sb[p, kc*512 + d] = WxT[kc*128 + p, d]
            wq_sb = wpool.tile([128, KC * DL], f16, name="wq_sb")
            wk_sb = wpool.tile([128, KC * DL], f16, name="wk_sb")
            wv_sb = wpool.tile([128, KC * DL], f16, name="wv_sb")
            # wo_sb[p, t*1024 + o] = WoT[t*128 + p, o]
            wo_sb = wpool.tile([128, DC * D], f16, name="wo_sb")
            # biases as [128, DC] (per-partition scalars per d-chunk) —
            # tiny, needed by the first projection evictions: queue first.
            bq_sb = bpool.tile([128, DC], f32, name="bq_sb")
            bk_sb = bpool.tile([128, DC], f32, name="bk_sb")
            nc.sync.dma_start(bq_sb[:], bq.ap().rearrange("(t p) -> p t", p=128))
            nc.sync.dma_start(bk_sb[:], bk.ap().rearrange("(t p) -> p t", p=128))

            # persistent activations
            qt_sb = [qkv_sb.tile([128, S], f16, name=f"qt{t}") for t in range(DC)]
            kt_sb = [qkv_sb.tile([128, S], f16, name=f"kt{t}") for t in range(DC)]
            v_sb = [qkv_sb.tile([128, HL * VW], f16, name=f"v{sc}") for sc in range(SC16)]
            ot_sb = [ot_pool.tile([128, S], f16, name=f"ot{t}") for t in range(DC)]

            # PSUM budget (8 banks): stq 2x[128,1024]f32 = 4, av 2x[65,512]
            # = 2, ps 2x[128,512] = 2.
            with (
                tc.tile_pool(name="psum", bufs=1, space="PSUM") as psum,
                tc.tile_pool(name="xt_pool", bufs=16) as xt_pool,
                tc.tile_pool(name="vsl_pool", bufs=6) as vsl_pool,
                tc.tile_pool(name="e_pool", bufs=6) as e_pool,
                tc.tile_pool(name="n_pool", bufs=1) as n_pool,
                tc.tile_pool(name="y_pool", bufs=2) as y_pool,
            ):

                # ---- PE warm-up: the HAM clock gate keeps the PE at 1.2GHz
                # until ~3.4us of sustained matmul activity.  The head is
                # DMA-bound anyway, so spin the array on a zero scratch tile
                # to hit 2.4GHz before the real projections arrive.
                scratch = bpool.tile([128, 512], f16, name="warm")
                nc.vector.memset(scratch[:], 0.0)
                for _ in range(24):
                    wps = psum.tile([128, 512], f32, name="ps", tag="ps", bufs=2)
                    nc.tensor.matmul(
                        wps[:],
                        lhsT=scratch[:, 0:128],
                        rhs=scratch[:],
                        start=True,
                        stop=True,
                        skip_group_check=True,
                    )

                def alloc_xt():
                    # one [128, S] tile per contraction chunk; DMA'd in
                    # 512-column pieces (one 3D transfer per column block)
                    # so early projection groups don't wait for the full S.
                    return [
                        xt_pool.tile([128, S], f16, name="xt", tag="xt")
                        for _ in range(KC)
                    ]

                def load_xt_cols(xt_c, x_dram, sc):
                    for kc in range(KC):
                        nc.sync.dma_start(
                            xt_c[kc][:, sc * 512 : (sc + 1) * 512],
                            x_dram.ap()[kc * 128 : (kc + 1) * 128, sc * 512 : (sc + 1) * 512],
                        )

                def vt_gather(sc):
                    vt = vsl_pool.tile([128, KC * 128], f16, name="vt", tag="vt")
                    nc.sync.dma_start(vt[:], xtvg.ap()[sc * 128 : (sc + 1) * 128, :])
                    return vt

                def qk_group(xt_c, w_sb, out_tiles, b_ap, dc, sc):
                    # one QT/KT projection group: out [d_local(part), 512 s]
                    ps = psum.tile([128, 512], f32, name="ps", tag="ps", bufs=2)
                    for kc in range(KC):
                        nc.tensor.matmul(
                            ps[:],
                            lhsT=w_sb[:, kc * DL + dc * 128 : kc * DL + dc * 128 + 128],
                            rhs=xt_c[kc][:, sc * 512 : (sc + 1) * 512],
                            start=(kc == 0),
                            stop=(kc == KC - 1),
                        )
                    # eviction + per-partition bias on DVE (keeps ACT free
                    # for attention exp)
                    nc.vector.tensor_scalar_add(
                        out_tiles[dc][:, sc * 512 : (sc + 1) * 512],
                        ps[:],
                        b_ap[:, dc : dc + 1],
                    )

                def v_proj(sc, vt):
                    # vt[p, kc*128+si] = XTv[kc*128+p, sc*128+si], gathered
                    # on the DVE DMA queue (prefetched; see emission below)
                    ps = psum.tile([128, 512], f32, name="ps", tag="ps", bufs=2)
                    for kc in range(KC):
                        nc.tensor.matmul(
                            ps[:],
                            lhsT=vt[:, kc * 128 : (kc + 1) * 128],
                            rhs=wv_sb[:, kc * DL : (kc + 1) * DL],
                            start=(kc == 0),
                            stop=(kc == KC - 1),
                        )
                    v3 = v_sb[sc][:].rearrange("p (h x) -> p h x", x=VW)
                    nc.vector.tensor_copy(
                        v3[:, :, 0:HD], ps[:].rearrange("p (h x) -> p h x", x=HD)
                    )
                    nc.vector.memset(v3[:, :, HD : HD + 1], 1.0)

                def attention(qq, pr, interleave=None):
                    q0 = qq * 512
                    h0, h1 = 2 * pr, 2 * pr + 1
                    av0 = psum.tile([VW, 512], f32, name="av", tag="av", bufs=2)
                    av1 = psum.tile([VW, 512], f32, name="av", tag="av", bufs=2)
                    for kc in range(SC16):
                        st = psum.tile([128, 1024], f32, name="st", tag="stq", bufs=2)
                        nc.tensor.matmul(
                            st[:, 0:512],
                            lhsT=kt_sb[pr][0:64, kc * 128 : (kc + 1) * 128],
                            rhs=qt_sb[pr][0:64, q0 : q0 + 512],
                            start=True,
                            stop=True,
                        )
                        nc.tensor.matmul(
                            st[:, 512:1024],
                            lhsT=kt_sb[pr][64:128, kc * 128 : (kc + 1) * 128],
                            rhs=qt_sb[pr][64:128, q0 : q0 + 512],
                            start=True,
                            stop=True,
                        )
                        e = e_pool.tile([128, 1024], f16, name="e", tag="e")
                        nc.scalar.activation(e[:], st[:], ACT.Exp, scale=0.125)
                        nc.tensor.matmul(
                            av0[:],
                            lhsT=v_sb[kc][:, h0 * VW : (h0 + 1) * VW],
                            rhs=e[:, 0:512],
                            start=(kc == 0),
                            stop=(kc == SC16 - 1),
                            skip_group_check=True,
                        )
                        nc.tensor.matmul(
                            av1[:],
                            lhsT=v_sb[kc][:, h1 * VW : (h1 + 1) * VW],
                            rhs=e[:, 512:1024],
                            start=(kc == 0),
                            stop=(kc == SC16 - 1),
                            skip_group_check=True,
                        )
                        # ride-along emission AFTER this chunk's work so the
                        # chunk stream outprioritizes it
                        if interleave is not None:
                            interleave(kc)
                    # Evict AV PSUM -> SBUF immediately (frees the banks for
                    # the next pair; normalization then runs out of SBUF off
                    # the PE critical path).
                    od0 = n_pool.tile([VW, 512], f32, name="od0", tag="od0")
                    od1 = n_pool.tile([VW, 512], f32, name="od1", tag="od1")
                    nc.vector.tensor_copy(od0[:], av0[:])
                    nc.vector.tensor_copy(od1[:], av1[:])
                    # normalize: O.T[hd, q] * (1 / denom[q]).  Denom rows sit
                    # at partition 64; SBUF DMA down to p0 -> reciprocal ->
                    # gpsimd partition-broadcast -> DVE multiply.
                    dd = n_pool.tile([1, 1024], f32, name="dd", tag="dd")
                    nc.sync.dma_start(dd[:, 0:512], od0[HD : HD + 1, :])
                    nc.sync.dma_start(dd[:, 512:1024], od1[HD : HD + 1, :])
                    r0 = n_pool.tile([1, 512], f32, name="r0", tag="r0")
                    r1 = n_pool.tile([1, 512], f32, name="r1", tag="r1")
                    nc.vector.reciprocal_approx_fast(r0[:], dd[:, 0:512])
                    nc.vector.reciprocal_approx_fast(r1[:], dd[:, 512:1024])
                    bc_a = n_pool.tile([64, 512], f32, name="bc_a", tag="bc_a")
                    bc_b = n_pool.tile([64, 512], f32, name="bc_b", tag="bc_b")
                    nc.gpsimd.partition_broadcast(bc_a[:], r0[:])
                    nc.gpsimd.partition_broadcast(bc_b[:], r1[:])
                    cols = slice(q0, q0 + 512)
                    nc.vector.tensor_mul(ot_sb[pr][0:64, cols], od0[0:64, :], bc_a[:])
                    tmp = n_pool.tile([64, 512], f16, name="tmp", tag="tmp")
                    nc.vector.tensor_mul(tmp[:], od1[0:64, :], bc_b[:])
                    nc.sync.dma_start(ot_sb[pr][64:128, cols], tmp[:])

                def out_proj(qq):
                    # 4 q-chunks of 128; 512-wide pieces on the ps tag so
                    # the attention scores keep both stq slots.  fp16 out.
                    for mc in range(qq * 4, qq * 4 + 4):
                        yt = y_pool.tile([128, D], f16, name="yt", tag="yt")
                        for pc in range(2):
                            pso = psum.tile([128, 512], f32, name="pso", tag="ps", bufs=2)
                            for t in range(DC):
                                nc.tensor.matmul(
                                    pso[:],
                                    lhsT=ot_sb[t][:, mc * 128 : (mc + 1) * 128],
                                    rhs=wo_sb[:, t * D + pc * 512 : t * D + (pc + 1) * 512],
                                    start=(t == 0),
                                    stop=(t == DC - 1),
                                    skip_group_check=True,
                                )
                            nc.vector.tensor_copy(yt[:, pc * 512 : (pc + 1) * 512], pso[:])
                        nc.sync.dma_start(y.ap()[mc * 128 : (mc + 1) * 128, :], yt[:])

                # ---- input DMA emission (sync queue, strict first-use
                # order; the critical path to the first exp is
                # wq+xtq(s0)+wk+xtk(s0) = 4MB ~ 12us).
                xtq_c = alloc_xt()
                xtk_c = alloc_xt()
                nc.sync.dma_start(
                    wq_sb[:].rearrange("p (kc d) -> p kc d", d=DL),
                    wqt.ap().rearrange("(kc p) d -> p kc d", p=128),
                )
                load_xt_cols(xtq_c, xtq, 0)
                nc.sync.dma_start(
                    wk_sb[:].rearrange("p (kc d) -> p kc d", d=DL),
                    wkt.ap().rearrange("(kc p) d -> p kc d", p=128),
                )
                load_xt_cols(xtk_c, xtk, 0)
                nc.sync.dma_start(
                    wv_sb[:].rearrange("p (kc d) -> p kc d", d=DL),
                    wvt.ap().rearrange("(kc p) d -> p kc d", p=128),
                )
                # 5 V-slice gathers up front (v_proj 0-4 fill the PE idle in
                # the DMA-bound head); later gathers are emitted lazily as
                # v_proj consumes slots so the slot-reuse dependency lands
                # after the consumer.
                vts = {sc: vt_gather(sc) for sc in range(5)}
                next_vt = [5]
                for sc in range(1, SC4):
                    load_xt_cols(xtk_c, xtk, sc)
                for sc in range(1, SC4):
                    load_xt_cols(xtq_c, xtq, sc)
                nc.sync.dma_start(
                    wo_sb[:].rearrange("p (t o) -> p t o", o=D),
                    wot.ap().rearrange("(t p) o -> p t o", p=128),
                )

                def v_proj_g(sc):
                    # demoted: the attention stream preempts; the scheduler
                    # fits each group just-in-time for its AV chunk.  The
                    # slot-refill gather stays at ambient priority (sync
                    # queue position).
                    with tc.high_priority(-500_000):
                        v_proj(sc, vts.pop(sc))
                    if next_vt[0] < SC16:
                        vts[next_vt[0]] = vt_gather(next_vt[0])
                        next_vt[0] += 1

                # ---- first pair's projections: q(sc0)/k(sc0) feed the
                # first scores; kt sc1-3 feed chunks 4-15 of every pr0
                # q-block (natural priority); v_proj rides; qt sc1-3 are
                # only needed from q-block 1 on.
                qk_group(xtq_c, wq_sb, qt_sb, bq_sb, 0, 0)
                qk_group(xtk_c, wk_sb, kt_sb, bk_sb, 0, 0)
                for sc in range(1, SC4):
                    qk_group(xtk_c, wk_sb, kt_sb, bk_sb, 0, sc)
                for sc in range(5):
                    v_proj_g(sc)
                with tc.high_priority(-750_000):
                    for sc in range(1, SC4):
                        qk_group(xtq_c, wq_sb, qt_sb, bq_sb, 0, sc)

                def v_interleave(kc):
                    sc = kc + 5
                    if sc < SC16:
                        v_proj_g(sc)

                # ---- pair-outer schedule.  Next pair's Q/K projections are
                # emitted after the pair's four q-blocks (lower scheduler
                # priority -> they fill PE slack inside the exp-paced
                # stream; kt first, the next pair's chunk sweep needs all of
                # it); out-projection rides the last pair per q-block.
                for pr in range(DC):
                    for qq in range(4):
                        inter = v_interleave if (pr == 0 and qq == 0) else None
                        attention(qq, pr, interleave=inter)
                        if pr == DC - 1:
                            with tc.high_priority(-1_000_000):
                                out_proj(qq)
                    if pr < DC - 1:
                        with tc.high_priority(-1_000_000):
                            for sc in range(SC4):
                                qk_group(xtk_c, wk_sb, kt_sb, bk_sb, pr + 1, sc)
                            for sc in range(SC4):
                                qk_group(xtq_c, wq_sb, qt_sb, bq_sb, pr + 1, sc)

    nc.compile()
    return nc


def get_program():
    global _PROGRAM
    if _PROGRAM is None:
        _PROGRAM = _build_program()
    return _PROGRAM


def make_in_maps(query, key, value, Wq, bq, Wk, bk, Wv, bv, Wo, bo):
    """Per-core input dicts. Core c: batch c//2, head-group c%2."""
    query = np.asarray(query, np.float32)
    key = np.asarray(key, np.float32)
    value = np.asarray(value, np.float32)
    xt = {}
    for b in range(B):
        xtv = value[b].T.astype(np.float16)  # [D, S]
        # xtvg[sc*128+p, kc*128+si] = xtv[kc*128+p, sc*128+si]
        xtvg = np.ascontiguousarray(
            xtv.reshape(KC, 128, SC16, 128).transpose(2, 1, 0, 3).reshape(S, D)
        )
        xt[b] = (
            np.ascontiguousarray(query[b].T.astype(np.float16)),
            np.ascontiguousarray(key[b].T.astype(np.float16)),
            xtvg,
        )
    wslices = {}
    for hg in range(2):
        sl = slice(hg * DL, (hg + 1) * DL)
        wslices[hg] = dict(
            wqt=np.ascontiguousarray(np.asarray(Wq, np.float32)[sl, :].T.astype(np.float16)),
            wkt=np.ascontiguousarray(np.asarray(Wk, np.float32)[sl, :].T.astype(np.float16)),
            wvt=np.ascontiguousarray(np.asarray(Wv, np.float32)[sl, :].T.astype(np.float16)),
            wot=np.ascontiguousarray(np.asarray(Wo, np.float32)[:, sl].T.astype(np.float16)),
            bq=np.ascontiguousarray(np.asarray(bq, np.float32)[sl]),
            bk=np.ascontiguousarray(np.asarray(bk, np.float32)[sl]),
        )
    in_maps = []
    for c in range(NCORES):
        b, hg = c // 2, c % 2
        m = dict(xtq=xt[b][0], xtk=xt[b][1], xtvg=xt[b][2])
        m.update(wslices[hg])
        in_maps.append(m)
    return in_maps


def combine_outputs(results, Wo, bo, bv):
    """Sum the two head-group partials per batch + host-side bias constant."""
    Wo = np.asarray(Wo, np.float32)
    bo = np.asarray(bo, np.float32)
    bv = np.asarray(bv, np.float32)
    const = bv @ Wo.T + bo  # [D]
    out = np.empty((B, S, D), np.float32)
    for b in range(B):
        out[b] = (
            results[2 * b]["y"].astype(np.float32)
            + results[2 * b + 1]["y"].astype(np.float32)
            + const
        )
    return out


def kernel(query, key, value, Wq, bq, Wk, bk, Wv, bv, Wo, bo):
    from concourse.bass_utils import run_bass_kernel_spmd

    nc = get_program()
    in_maps = make_in_maps(query, key, value, Wq, bq, Wk, bk, Wv, bv, Wo, bo)
    res = run_bass_kernel_spmd(nc, in_maps, core_ids=list(range(NCORES)))
    return combine_outputs(res.results, Wo, bo, bv)
